# revision 38
# baseline (speedup 1.0000x reference)
"""Trainium2 Bass kernel for ConvMHSA (B=16, C=512, H=W=32, 8 heads).

Data-parallel over batch: each of the 8 NeuronCores processes 2 batches.
All matmuls run in bf16 (full PE rate at any moving-dim size); inputs and
weights are pre-swizzled/pre-cast on the host so each SBUF tile fills with
one large strided DMA (row c_lo holds all 4 contraction tiles).

Layout strategy per batch (xf = x reshaped to (C=512, N=1024)):
  - Q, K projected into (C, N) bf16 (heads stacked on partitions); the
    weight columns are host-reordered [q_p|k_p per pair | v] so the first
    pair's columns arrive in one aligned DMA.
  - V projected TRANSPOSED to vT (N, hd) bf16 per head, with a ones column
    appended -> the PV matmul emits softmax denominators Z for free.
  - Scores transposed: S^T[m, n] per head, two heads packed on the PE via
    tile_position row groups.
  - exp fused into the PSUM->SBUF copy on ScalarE (scale=1/8 folded in),
    output bf16. ScalarE is the bottleneck engine (~134 us busy); the
    schedule keeps it saturated (<1.5 us mid-stream idle).
  - PV in nT orientation: for each 128-token n-chunk, psum accumulates
    lhsT=E-chunk x rhs=[vT|1] over the 8 m-tiles (65-row moving dim);
    columns 64/129 hold Z per partition.
  - Normalization: reciprocal of the Z columns -> native per-partition
    tensor_scalar_mul (no partition broadcast), bf16 attnT.
  - attnT -> attn: mid-stream via the DMA-engine xbar transpose (bounced
    through DRAM; SBUF-source xbar reads are mis-addressed on hw), in the
    drain via PE transposes + ScalarE copies (ScalarE is idle there).
  - Output projection bf16 with gamma folded into the weights on the host;
    residual added from the bf16 x; y staged bf16 and cast to f32 on host.

Emission software-pipelines at head-pair granularity: stretch (b, p) emits
pair p's scores/exp interleaved with pair p-1's PV/norm plus a balanced
budget of projection/output filler matmuls so the PE never outruns the
ScalarE exp stream and never stalls on PSUM. Dummy warm-up matmuls at the
start hold the PE p-state at full rate until the first input DMAs land.
"""

import os
import sys

sys.path.insert(0, "/opt/trn_rl_repo")

import numpy as np

B, C, H, W = 16, 512, 32, 32
HEADS = 8
HD = C // HEADS          # 64
N = H * W                # 1024
NCORES = 8
NB = B // NCORES         # batches per core = 2
KT = C // 128            # 4 contraction tiles of 128
NCH = N // 512           # 2 moving chunks of 512
MT = N // 128            # 8 m-tiles / n-chunks
NPAIR = HEADS // 2       # 4 head-pairs

_cache = {}


def _build_nc(debug=False):
    import concourse.bass as bass
    import concourse.tile as tile
    import concourse.mybir as mybir
    from concourse import bacc

    F32 = mybir.dt.float32
    F32R = mybir.dt.float32r
    BF16 = mybir.dt.bfloat16
    EXP = mybir.ActivationFunctionType.Exp
    COPY = mybir.ActivationFunctionType.Copy
    IDENT = mybir.ActivationFunctionType.Identity

    nc = bacc.Bacc("TRN2", target_bir_lowering=False, debug=False,
                   num_devices=NCORES)

    # xs/wqkvT/woTg are pre-swizzled on the host so each SBUF tile fills
    # with one large strided DMA: row c_lo holds all KT contraction tiles.
    xs = nc.dram_tensor("xs", (NB, 128, KT * N), BF16, kind="ExternalInput").ap()
    wqkvT = nc.dram_tensor("wqkvT", (128, KT * 3 * C), BF16, kind="ExternalInput").ap()
    bqkv_col = nc.dram_tensor("bqkv_col", (128, 12), F32, kind="ExternalInput").ap()
    bqkv_row = nc.dram_tensor("bqkv_row", (1, 3 * C), F32R, kind="ExternalInput").ap()
    woTg = nc.dram_tensor("woTg", (C, C), BF16, kind="ExternalInput").ap()
    bog_col = nc.dram_tensor("bog_col", (128, KT), F32, kind="ExternalInput").ap()
    identD = nc.dram_tensor("identD", (128, 128), BF16, kind="ExternalInput").ap()
    y = nc.dram_tensor("y", (NB, C, N), BF16, kind="ExternalOutput").ap()
    if debug:
        dq = nc.dram_tensor("dq", (128, N), BF16, kind="ExternalOutput").ap()
        dk = nc.dram_tensor("dk", (128, N), BF16, kind="ExternalOutput").ap()
        de = nc.dram_tensor("de", (2, 128, N), BF16, kind="ExternalOutput").ap()
        dvt = nc.dram_tensor("dvt", (128, HEADS * (HD + 1)), BF16,
                             kind="ExternalOutput").ap()
        dat = nc.dram_tensor("dat", (128, N), BF16, kind="ExternalOutput").ap()
        daf = nc.dram_tensor("daf", (128, N), BF16, kind="ExternalOutput").ap()

    with tile.TileContext(nc) as tc:
        with tc.tile_pool(name="const", bufs=1) as const, \
             tc.tile_pool(name="xfp", bufs=2) as xfp, \
             tc.tile_pool(name="qkp", bufs=1) as qkp, \
             tc.tile_pool(name="epool", bufs=40) as epool, \
             tc.tile_pool(name="vtp", bufs=2) as vtp, \
             tc.tile_pool(name="atp", bufs=3) as atp, \
             tc.tile_pool(name="afp", bufs=2) as afp, \
             tc.tile_pool(name="small", bufs=4) as small, \
             tc.tile_pool(name="rzp", bufs=8) as rzp, \
             tc.tile_pool(name="spool", bufs=2, space="PSUM") as spool, \
             tc.tile_pool(name="bank1", bufs=4, space="PSUM") as bank1, \
             tc.tile_pool(name="zdram", bufs=6, space="DRAM") as zdram:

            # ---- weights/x: big strided DMAs, first-needed columns first ----
            wq_t = const.tile([128, KT * 3 * C], BF16, tag="wq", name="wq_t")
            wq3 = wq_t.rearrange("p (k o) -> p k o", k=KT)
            wsrc = wqkvT.rearrange("p (k o) -> p k o", k=KT)
            # per-kc column layout: [q_p0 k_p0 q_p1 k_p1 ... | v]
            bq_col = const.tile([128, 12], F32, tag="bqcol")
            nc.sync.dma_start(out=bq_col, in_=bqkv_col)
            nc.sync.dma_start(out=wq3[:, :, 0:256], in_=wsrc[:, :, 0:256])

            xf_t0 = xfp.tile([128, KT * N], BF16, tag="xf", name="xf_t")
            xf3 = xf_t0.rearrange("p (k n) -> p k n", k=KT)
            x0src = xs[0].rearrange("p (k n) -> p k n", k=KT)
            nc.sync.dma_start(out=xf3[:, :, 0:512], in_=x0src[:, :, 0:512])
            nc.sync.dma_start(out=xf3[:, :, 512:N], in_=x0src[:, :, 512:N])
            # v columns, then the remaining q/k columns
            nc.sync.dma_start(out=wq3[:, :, 1024:1536], in_=wsrc[:, :, 1024:1536])
            ident = const.tile([128, 128], BF16, tag="ident", name="ident")
            nc.sync.dma_start(out=ident, in_=identD)
            nc.sync.dma_start(out=wq3[:, :, 256:1024], in_=wsrc[:, :, 256:1024])
            wo = []
            for kc in range(KT):
                t = const.tile([128, C], BF16, tag=f"wo{kc}", name=f"wo{kc}")
                nc.sync.dma_start(out=t, in_=woTg[128 * kc:128 * (kc + 1), :])
                wo.append(t)
            wq = [wq_t[:, 1536 * kc:1536 * (kc + 1)] for kc in range(KT)]
            bo_col = const.tile([128, KT], F32, tag="bocol")
            nc.sync.dma_start(out=bo_col, in_=bog_col)
            bv_bc = const.tile([128, C], F32, tag="bvbc")
            bv_src = bass.AP(tensor=bqkv_row.tensor, offset=2 * C,
                             ap=[[0, 128], [1, C]])
            nc.sync.dma_start(out=bv_bc, in_=bv_src.bitcast(F32))
            xf0 = [xf_t0[:, N * kc:N * (kc + 1)] for kc in range(KT)]

            # PE p-state warm-up: matmuls on an uninitialized scratch tile
            # while the first DMAs land. Results are never read.
            warm = const.tile([128, 512], BF16, tag="warm", name="warm")
            nc.gpsimd.memset(warm, 1.0)
            for i in range(10):
                wps = bank1.tile([128, 512], F32, tag="bank1", name="wps")
                nc.tensor.matmul(wps, warm[:, 0:128], warm,
                                 start=True, stop=True)

            def load_xf(b):
                t = xfp.tile([128, KT * N], BF16, tag="xf", name="xf_t")
                t3 = t.rearrange("p (k n) -> p k n", k=KT)
                src = xs[b].rearrange("p (k n) -> p k n", k=KT)
                nc.sync.dma_start(out=t3[:, :, 0:512], in_=src[:, :, 0:512])
                nc.sync.dma_start(out=t3[:, :, 512:N], in_=src[:, :, 512:N])
                return [t[:, N * kc:N * (kc + 1)] for kc in range(KT)]

            # ---- building blocks ----
            def proj_qk_chunk(xf, pair, which, nch, dest):
                """One (q|k, nch) chunk of the pair projection: 2048 rows."""
                ot = pair if which == "q" else KT + pair
                col = 256 * pair + (0 if which == "q" else 128)
                ps = bank1.tile([128, 512], F32, tag="bank1", name="ps")
                for kc in range(KT):
                    nc.tensor.matmul(
                        ps,
                        wq[kc][:, col:col + 128],
                        xf[kc][:, 512 * nch:512 * (nch + 1)],
                        start=(kc == 0), stop=(kc == KT - 1))
                nc.vector.tensor_scalar_add(
                    out=dest[:, 512 * nch:512 * (nch + 1)],
                    in0=ps, scalar1=bq_col[:, ot:ot + 1])

            def proj_vT_mtile(xf, vt, mt):
                """vT m-tile: (128 m, 8 heads, hd+1) bf16; 2048 rows."""
                ps = bank1.tile([128, 512], F32, tag="bank1", name="ps")
                for kc in range(KT):
                    nc.tensor.matmul(
                        ps,
                        xf[kc][:, 128 * mt:128 * (mt + 1)],
                        wq[kc][:, 2 * C:3 * C],
                        start=(kc == 0), stop=(kc == KT - 1))
                nc.vector.tensor_add(
                    out=vt[:, :, 0:HD],
                    in0=ps.rearrange("p (a b) -> p a b", a=HEADS),
                    in1=bv_bc.rearrange("p (a b) -> p a b", a=HEADS))
                nc.gpsimd.memset(vt[:, :, HD:HD + 1], 1.0)

            def scores_exp(qt, kt_, mt, ebuf):
                """Scores+exp for both heads of the pair at m-tile mt."""
                for par in range(2):
                    s = spool.tile([128, N], F32, tag="s", name="s")
                    lo = 64 * par
                    for nch in range(NCH):
                        nc.tensor.matmul(
                            s[:, 512 * nch:512 * (nch + 1)],
                            kt_[lo:lo + 64, 128 * mt:128 * (mt + 1)],
                            qt[lo:lo + 64, 512 * nch:512 * (nch + 1)],
                            start=True, stop=True,
                            tile_position=(lo, 0))
                    e = epool.tile([128, N], BF16, tag="e", name="e")
                    nc.scalar.activation(out=e, in_=s, func=EXP, scale=0.125)
                    ebuf[par].append(e)

            def pv_nchunk(eh, vt, pair, c, aT, act_mul=False):
                """PV + norm for n-chunk c of `pair`: attnT[n, cpair] bf16
                written into aT[:, 128c:128c+128]."""
                P = bank1.tile([128, 512], F32, tag="bank1", name="P")
                for h in range(2):
                    for j in range(MT):
                        nc.tensor.matmul(
                            P[:, 65 * h:65 * (h + 1)],
                            eh[h][j][:, 128 * c:128 * (c + 1)],
                            vt[j][:, 2 * pair + h, :],
                            start=(j == 0), stop=(j == MT - 1))
                rz = rzp.tile([128, 2], F32, tag="rz", name="rz")
                zcols = bass.AP(tensor=P.tensor, offset=P.offset + HD,
                                ap=[list(P.ap[0]), [65, 2]])
                nc.vector.reciprocal_approx_fast(out=rz, in_=zcols)
                for h in range(2):
                    dst = aT[:, 128 * c + 64 * h:128 * c + 64 * (h + 1)]
                    if act_mul and h == 1:
                        nc.scalar.activation(
                            out=dst, in_=P[:, 65 * h:65 * h + 64],
                            func=COPY, scale=rz[:, h:h + 1])
                    else:
                        nc.vector.tensor_scalar_mul(
                            out=dst, in0=P[:, 65 * h:65 * h + 64],
                            scalar1=rz[:, h:h + 1])

            def transpose_half(aT, af, half, engine=None):
                """attnT[n, cpair] -> attn[cpair, n] for j-chunks of one
                512-token half via the xbar DMA transpose. The xbar path
                mis-addresses SBUF sources on hardware, so bounce through a
                DRAM scratch tile (the validated DRAM->SBUF pattern)."""
                eng = engine or nc.sync
                tz = zdram.tile([128, 512], BF16, tag="tz", name="tz")
                eng.dma_start(out=tz, in_=aT[:, 512 * half:512 * (half + 1)])
                eng.dma_start_transpose(
                    out=af.rearrange("p (j n) -> p j n", j=MT)[:, 4 * half:4 * (half + 1), :],
                    in_=tz)

            def out_proj_chunk(xf, attn, b, ot, nch, eng=None):
                ps = bank1.tile([128, 512], F32, tag="bank1", name="ps")
                for kc in range(KT):
                    nc.tensor.matmul(
                        ps,
                        wo[kc][:, 128 * ot:128 * (ot + 1)],
                        attn[kc][:, 512 * nch:512 * (nch + 1)],
                        start=(kc == 0), stop=(kc == KT - 1))
                osb = small.tile([128, 512], BF16, tag="osb")
                (eng or nc.vector).scalar_tensor_tensor(
                    out=osb, in0=ps, scalar=bo_col[:, ot:ot + 1],
                    in1=xf[ot][:, 512 * nch:512 * (nch + 1)],
                    op0=mybir.AluOpType.add, op1=mybir.AluOpType.add)
                nc.sync.dma_start(
                    out=y[b, 128 * ot:128 * (ot + 1),
                          512 * nch:512 * (nch + 1)],
                    in_=osb)

            # ---- static state ----
            xf_all = {0: xf0}
            q_all, k_all, e_all, vT_all, aT_all, af_all = {}, {}, {}, {}, {}, {}

            def get_qk(b, pr):
                if (b, pr) not in q_all:
                    q_all[b, pr] = qkp.tile([128, N], BF16, tag=f"q{pr}",
                                            name=f"q{pr}")
                    k_all[b, pr] = qkp.tile([128, N], BF16, tag=f"k{pr}",
                                            name=f"k{pr}")
                return q_all[b, pr], k_all[b, pr]

            def mkjob(fn, *a):
                return lambda: fn(*a)

            def qk_jobs(b, pr):
                xf = xf_all[b]
                jobs = []
                for nch in range(NCH):
                    for w in ("q", "k"):
                        q, k = get_qk(b, pr)
                        dest = q if w == "q" else k
                        jobs.append(mkjob(proj_qk_chunk, xf, pr, w, nch, dest))
                return jobs

            def vt_jobs(b):
                vT_all[b] = [vtp.tile([128, HEADS, HD + 1], BF16,
                                      tag=f"vT{mt}", name=f"vT{mt}")
                             for mt in range(MT)]
                xf = xf_all[b]
                return [mkjob(proj_vT_mtile, xf, vT_all[b][mt], mt)
                        for mt in range(MT)]

            def op_jobs(b):
                return [mkjob(out_proj_chunk, xf_all[b], af_all[b], b,
                              ot, nch)
                        for nch in range(NCH) for ot in range(KT)]

            def prev_of(b, p):
                if (b, p) == (0, 0):
                    return None
                return (b, p - 1) if p > 0 else (b - 1, NPAIR - 1)

            def stretch(b, p, fillers):
                """Emit scores+exp for pair (b, p), interleaved with PV+norm
                of the previous pair and the given filler jobs."""
                prev = prev_of(b, p)
                if prev is not None:
                    pb, pp = prev
                    aT = atp.tile([128, N], BF16, tag="aT", name="aT")
                    aT_all[prev] = aT
                e_all[b, p] = ebuf = [[], []]
                qt, kt_ = get_qk(b, p)
                nf = len(fillers)
                for mt in range(MT):
                    if prev is None:
                        scores_exp(qt, kt_, mt, ebuf)
                    for i in range(nf * mt // MT, nf * (mt + 1) // MT):
                        fillers[i]()
                    if prev is not None:
                        scores_exp(qt, kt_, mt, ebuf)
                    if prev is not None:
                        pv_nchunk(e_all[pb, pp], vT_all[pb], pp, mt, aT)
                        if mt == 3 or mt == 7:
                            transpose_half(aT, af_all[pb][pp], mt // 4)
                for i in range(nf * MT // MT, nf):
                    fillers[i]()

            def drain(b):
                """PV+norm+transpose for the final pair of batch b, then the
                batch's output projection, pipelined by halves."""
                pb, pp = b, NPAIR - 1
                aT = atp.tile([128, N], BF16, tag="aT", name="aT")
                aT_all[pb, pp] = aT
                ops = op_jobs(b)
                def pe_transpose_half(half):
                    tp = spool.tile([128, 2 * N], BF16, tag="s", name="tp")
                    for ci in range(4):
                        c = 4 * half + ci
                        nc.tensor.transpose(
                            out=tp[:, 128 * ci:128 * (ci + 1)],
                            in_=aT[:, 128 * c:128 * (c + 1)],
                            identity=ident)
                        nc.scalar.activation(
                            out=af_all[pb][pp][:, 128 * c:128 * (c + 1)],
                            in_=tp[:, 128 * ci:128 * (ci + 1)],
                            func=COPY)
                for c in range(4):
                    pv_nchunk(e_all[pb, pp], vT_all[pb], pp, c, aT,
                              act_mul=True)
                pe_transpose_half(0)
                for c in range(4, MT):
                    pv_nchunk(e_all[pb, pp], vT_all[pb], pp, c, aT,
                              act_mul=True)
                pe_transpose_half(1)
                xf = xf_all[b]
                for nch in range(NCH):
                    for op2 in range(2):          # ot pairs (0,1) and (2,3)
                        osb2 = small.tile([128, 1024], BF16, tag="osb2",
                                          name="osb2")
                        for oti in range(2):
                            ot = 2 * op2 + oti
                            ps = bank1.tile([128, 512], F32, tag="bank1",
                                            name="ps")
                            for kc in range(KT):
                                nc.tensor.matmul(
                                    ps,
                                    wo[kc][:, 128 * ot:128 * (ot + 1)],
                                    af_all[b][kc][:, 512 * nch:512 * (nch + 1)],
                                    start=(kc == 0), stop=(kc == KT - 1))
                            tmpo = small.tile([128, 512], BF16, tag="tmpo",
                                              name="tmpo")
                            nc.scalar.activation(
                                out=tmpo, in_=ps, func=IDENT,
                                bias=bo_col[:, ot:ot + 1])
                            nc.vector.tensor_add(
                                out=osb2[:, 512 * oti:512 * (oti + 1)],
                                in0=tmpo,
                                in1=xf[ot][:, 512 * nch:512 * (nch + 1)])
                        nc.sync.dma_start(
                            out=y[b, 256 * op2:256 * (op2 + 1),
                                  512 * nch:512 * (nch + 1)]
                            .rearrange("(o p) n -> p o n", o=2),
                            in_=osb2.rearrange("p (o n) -> p o n", o=2))

            # ---- schedule ----
            for b in range(NB):
                af_all[b] = [afp.tile([128, N], BF16, tag=f"attn{t}",
                                      name=f"attn{t}") for t in range(KT)]

            for j in qk_jobs(0, 0):
                j()
            v0rest = vt_jobs(0)

            vt1_jobs = None
            op_jobs_b0 = None
            qk12_rest = None
            for b in range(NB):
                for p in range(NPAIR):
                    if b == 0 and p == 1:
                        xf_all[1] = load_xf(1)
                        vt1_jobs = vt_jobs(1)
                    if b == 1 and p == 0:
                        op_jobs_b0 = op_jobs(0)
                    if b == 0 and p == 0:
                        f = v0rest + qk_jobs(0, 1)
                    elif b == 0 and p == 1:
                        f = qk_jobs(0, 2) + vt1_jobs[0:2]
                    elif b == 0 and p == 2:
                        f = qk_jobs(0, 3) + vt1_jobs[2:4]
                    elif b == 0 and p == 3:
                        f = qk_jobs(1, 0) + vt1_jobs[4:8]
                    elif b == 1 and p == 0:
                        qk12 = qk_jobs(1, 2)
                        f = qk_jobs(1, 1) + qk12[0:2]
                        qk12_rest = qk12[2:4]
                    elif b == 1 and p == 1:
                        f = qk12_rest + qk_jobs(1, 3) + op_jobs_b0[0:2]
                    elif b == 1 and p == 2:
                        f = op_jobs_b0[2:8]
                    else:
                        f = []
                    stretch(b, p, f)
                    if debug and (b, p) == (0, 0):
                        nc.sync.dma_start(out=dq, in_=q_all[0, 0])
                        nc.sync.dma_start(out=dk, in_=k_all[0, 0])
                        for par in range(2):
                            nc.sync.dma_start(out=de[par],
                                              in_=e_all[0, 0][par][0])
                        nc.sync.dma_start(
                            out=dvt,
                            in_=vT_all[0][0].rearrange("p a b -> p (a b)"))
                    if debug and (b, p) == (0, 1):
                        nc.sync.dma_start(out=dat, in_=aT_all[0, 0])
                        nc.sync.dma_start(out=daf, in_=af_all[0][0])
            drain(NB - 1)

    nc.compile()
    return nc


def kernel(x, qkv_w, qkv_b, out_w, out_b, gamma):
    import ml_dtypes
    from concourse.bass_utils import run_bass_kernel_spmd

    x = np.asarray(x, dtype=np.float32)
    qkv_w = np.asarray(qkv_w, dtype=np.float32)
    qkv_b = np.asarray(qkv_b, dtype=np.float32)
    out_w = np.asarray(out_w, dtype=np.float32)
    out_b = np.asarray(out_b, dtype=np.float32)
    gamma = np.asarray(gamma, dtype=np.float32)

    if "nc" not in _cache:
        _cache["nc"] = _build_nc()
    nc = _cache["nc"]

    xf = x.reshape(B, C, N)
    # swizzle: row c_lo holds all KT contraction tiles side by side
    xs_r = np.ascontiguousarray(
        xf.reshape(B, KT, 128, N).transpose(0, 2, 1, 3)
        .reshape(B, 128, KT * N).astype(ml_dtypes.bfloat16))
    wT = qkv_w.T.reshape(KT, 128, 3 * C).transpose(1, 0, 2)  # (128, KT, 3C)
    qcols = wT[:, :, 0:C].reshape(128, KT, 4, 128)
    kcols = wT[:, :, C:2 * C].reshape(128, KT, 4, 128)
    qk_il = np.stack([qcols, kcols], axis=3)      # (128, KT, 4, 2, 128)
    wql = np.concatenate([qk_il.reshape(128, KT, 2 * C),
                          wT[:, :, 2 * C:3 * C]], axis=2)
    wqkvT = np.ascontiguousarray(
        wql.reshape(128, KT * 3 * C).astype(ml_dtypes.bfloat16))
    bq_col = np.ascontiguousarray(qkv_b.reshape(12, 128).T)  # (128, 12)
    bq_row = np.ascontiguousarray(qkv_b.reshape(1, 3 * C))
    g = gamma.reshape(-1)[0]
    woTg = np.ascontiguousarray((g * out_w).T.astype(ml_dtypes.bfloat16))
    bog_col = np.ascontiguousarray((g * out_b).reshape(KT, 128).T)
    identE = np.ascontiguousarray(np.eye(128, dtype=np.float32)
                                  .astype(ml_dtypes.bfloat16))

    in_maps = []
    for c in range(NCORES):
        in_maps.append({
            "xs": np.ascontiguousarray(xs_r[NB * c:NB * (c + 1)]),
            "wqkvT": wqkvT,
            "bqkv_col": bq_col,
            "bqkv_row": bq_row,
            "woTg": woTg,
            "bog_col": bog_col,
            "identD": identE,
        })

    trace = bool(int(os.environ.get("KERNEL_TRACE", "0")))
    try:
        res = run_bass_kernel_spmd(nc, in_maps, core_ids=list(range(NCORES)),
                                   trace=trace)
    except ModuleNotFoundError:
        # NTFF profiling hooks unavailable under this axon client
        res = run_bass_kernel_spmd(nc, in_maps, core_ids=list(range(NCORES)),
                                   trace=False)
    _cache["last_result"] = res

    out = np.concatenate([np.asarray(res.results[c]["y"], dtype=np.float32)
                          for c in range(NCORES)], axis=0)
    return out.reshape(B, C, H, W)
